# revision 26
# baseline (speedup 1.0000x reference)
"""Benes butterfly network (12 layers, N=4096) on 8 Trainium2 NeuronCores.

Self-contained: takes full inputs, shards batch across 8 cores, runs a
Bass/Tile kernel per core, gathers the full output.

Math: reference layer k is a butterfly with span 2^k:
    h[:, j] <- A_k[j] * h[:, j] + B_k[j] * h[:, j ^ 2^k]
(A_k/B_k extracted from the sparse COO (values, idx_in, idx_out)).

Final design (the baseline was DMA-bound: 33.6MB/core at ~290GB/s; this
version moves 21MB by casting all wire data to bf16 on the host, and is
structured around measured TRN2 op costs — stt 747ns/tile,
tensor_scalar-with-AP-scalar 1232ns, ACT copy 687ns, warm N=512 matmul
219ns — which rule out element-wise implementations of more than one
butterfly pass):
  - Layers 0..8 composed on the host into dense 128x128 block matrices
    (mst, bf16) with layer 9's self-scale A9 folded in:
      p1[t] = sum_{ji} M[t, t^ji] @ H0[t^ji]     (PE, N=512, fp32 psum)
  - L9 partner via the ratio trick, one DVE stt pass; partner pairs
    (t, t^4) share one ACT evacuation so each stt reads one PSUM + one
    SBUF operand:  H9[t] = p1[t] + (B9[t]/A9[t^4]) * p1[t^4]
  - L10+L11 fused into the PE out-transpose (phase 2): per quad
    {q, q+8, q+16, q+24}, psum[b, 4*128] accumulates 4 matmuls with
    host-built diagonal-block moving matrices (movd, bf16).
  - Residue pipelining: out tile d depends only on tiles t = d (mod 4);
    tiles processed per residue class so residue-0 stores overlap
    residue-1+ loads; emission interleaved (phase1(r+1) ahead of
    phase2(r)) so no engine queue ever stalls on another engine's chain.
"""
import os
import numpy as np
import ml_dtypes

N = 4096
BATCH = 4096
NLAYERS = 12
NCORES = 8
BSH = BATCH // NCORES      # 512 batch rows per core
T = N // 128               # 32 column tiles

_PROGRAM_CACHE = {}
LAST_EXEC_NS = None


def _extract_ab(values, idx_in, idx_out):
    """Per-layer butterfly coefficients A[k], B[k] (float64 [L, N])."""
    v = np.asarray(values, np.float64)
    ii = np.asarray(idx_in, np.int64)
    io = np.asarray(idx_out, np.int64)
    L, nnz = v.shape
    n = nnz // 2
    A = np.zeros((L, n))
    B = np.zeros((L, n))
    for k in range(L):
        s = 1 << k
        self_m = ii[k] == io[k]
        part_m = ii[k] == (io[k] ^ s)
        if not np.all(self_m | part_m):
            raise ValueError(f"layer {k}: unexpected sparse index structure")
        np.add.at(A[k], io[k][self_m], v[k][self_m])
        np.add.at(B[k], io[k][part_m], v[k][part_m])
    return A, B


def _clamp(a):
    return np.where(np.abs(a) < 1e-12, 1e-12, a)


# Residue-major tile ordering: seq[r*8 + m] = 4*m + r
_TSEQ = [4 * m + r for r in range(4) for m in range(8)]


def _host_precompute(values, idx_in, idx_out):
    A, B = _extract_ab(values, idx_in, idx_out)
    Ab = A.reshape(NLAYERS, T, 128)
    Bb = B.reshape(NLAYERS, T, 128)
    j = np.arange(128)

    # Dense composition of within-block layers 0..6, one 128x128 per tile.
    S = [np.eye(128) for _ in range(T)]
    for k in range(7):
        s = 1 << k
        for t in range(T):
            W = np.zeros((128, 128))
            W[j, j] = Ab[k, t]
            W[j, j ^ s] = Bb[k, t]
            S[t] = W @ S[t]
    # Cross-block layers 7, 8 (tile distances 1, 2): dict src_tile -> 128x128
    Sd = [{t: S[t]} for t in range(T)]
    for k in (7, 8):
        d = 1 << (k - 7)
        newS = []
        for t in range(T):
            out = {}
            for src, M in Sd[t].items():
                out[src] = Ab[k, t][:, None] * M
            for src, M in Sd[t ^ d].items():
                out[src] = out.get(src, 0) + Bb[k, t][:, None] * M
            newS.append(out)
        Sd = newS

    A9 = _clamp(Ab[9])

    # mst (bf16, residue-major tile order): block (t, ji) holds
    # (diag(A9[t]) @ Sd[t][t^ji]).T so matmul computes M @ H0.
    mst = np.zeros((128, T * 512), np.float32)
    for si, t in enumerate(_TSEQ):
        assert set(Sd[t].keys()) == {t, t ^ 1, t ^ 2, t ^ 3}
        for ji in range(4):
            M = A9[t][:, None] * Sd[t][t ^ ji]
            mst[:, si * 512 + ji * 128: si * 512 + (ji + 1) * 128] = (
                M.T.astype(np.float32)
            )

    # L9 partner ratio scales rB9[t] = B9[t] / A9[t^4]  (fp32 [128, T])
    tabs = np.zeros((128, T), np.float64)
    for t in range(T):
        tabs[:, t] = Bb[9, t] / A9[t ^ 4]

    # out-transpose movings, quad-major: for quad q, slot si (src s=q+8*si),
    # block k holds diag(c[q+8k <- s]) where c are the fused L10*L11
    # coefficients acting on H9 (post-L9 state)
    movd = np.zeros((128, T * 512), np.float32)
    for s in range(T):
        q = s & 7
        si = s >> 3
        for k in range(4):
            d = q + 8 * k
            if s == d:
                c = Ab[11, d] * Ab[10, d]
            elif s == (d ^ 8):
                c = Ab[11, d] * Bb[10, d]
            elif s == (d ^ 16):
                c = Bb[11, d] * Ab[10, d ^ 16]
            else:  # s == d ^ 24
                c = Bb[11, d] * Bb[10, d ^ 16]
            movd[j, q * 2048 + si * 512 + k * 128 + j] = c.astype(np.float32)

    return (
        mst.astype(ml_dtypes.bfloat16),
        tabs.astype(np.float32),
        movd.astype(ml_dtypes.bfloat16),
    )


def _build_program():
    import concourse.bass as bass
    import concourse.mybir as mybir
    import concourse.tile as tile
    from concourse import bacc

    f32 = mybir.dt.float32
    bf16 = mybir.dt.bfloat16
    mult = mybir.AluOpType.mult
    add = mybir.AluOpType.add

    nc = bacc.Bacc("TRN2", target_bir_lowering=False, debug=False)
    xT_ap = nc.dram_tensor("xT", [N, BSH], bf16, kind="ExternalInput").ap()
    mst_ap = nc.dram_tensor("mst", [128, T * 512], bf16, kind="ExternalInput").ap()
    tabs_ap = nc.dram_tensor("tabs", [128, T], f32, kind="ExternalInput").ap()
    mov_ap = nc.dram_tensor("movd", [128, T * 512], bf16, kind="ExternalInput").ap()
    out_ap = nc.dram_tensor("out", [BSH, N], f32, kind="ExternalOutput").ap()

    with tile.TileContext(nc) as tc:
        with (
            tc.tile_pool(name="const", bufs=1) as constp,
            tc.tile_pool(name="h0", bufs=8) as h0p,
            tc.tile_pool(name="mstp", bufs=8) as mstp,
            tc.tile_pool(name="chain", bufs=56) as chainp,
            tc.tile_pool(name="mov", bufs=8) as movp,
            tc.tile_pool(name="piece", bufs=6) as piecep,
            tc.tile_pool(name="ps1", bufs=4, space="PSUM") as psp1,
            tc.tile_pool(name="ps2", bufs=4, space="PSUM") as psp2,
        ):
            # Queued DMAs share bandwidth round-robin (NOT FIFO), so issuing
            # everything upfront makes all transfers finish late together.
            # Only the first-needed loads go out now; the rest are issued
            # from the ACT engine's queue between its PSUM evacuations,
            # which block on matmul progress and therefore pace the issues.
            tabs = constp.tile([128, T], f32, name="tabs")
            nc.sync.dma_start(tabs[:], tabs_ap[:])

            def rb9col(t):
                return tabs[:, t:t + 1]

            # mst chunks are [128, 2048] halves: (r, half) covers tiles
            # m = 4*half .. 4*half+3 of residue r
            msth = {}

            def issue_mst(r, half, eng=None, split=False):
                tile_ = mstp.tile(
                    [128, 2048], bf16, tag="mst", name=f"mst_{r}{'ab'[half]}"
                )
                base = r * 4096 + half * 2048
                e = eng or nc.scalar
                if split:
                    # two quarter DMAs: the first matmul pair only waits on
                    # the m=0,1 quarter (256KB instead of 512KB)
                    e.dma_start(tile_[:, 0:1024], mst_ap[:, base:base + 1024])
                    e.dma_start(
                        tile_[:, 1024:2048], mst_ap[:, base + 1024:base + 2048]
                    )
                else:
                    e.dma_start(tile_[:], mst_ap[:, base:base + 2048])
                msth[(r, half)] = tile_

            def mst_slice(r, m, ji):
                tile_ = msth[(r, m // 4)]
                mm = m % 4
                return tile_[:, mm * 512 + ji * 128:mm * 512 + (ji + 1) * 128]

            H0cat = {}

            def issue_h0(kb, eng):
                h0c = h0p.tile([128, 2048], bf16, tag="h0", name=f"h0c_{kb}")
                src = xT_ap[kb * 512:(kb + 1) * 512, :].rearrange(
                    "(lt p) b -> p lt b", lt=4, p=128
                )
                eng.dma_start(h0c[:].rearrange("p (lt b) -> p lt b", lt=4), src)
                H0cat[kb] = h0c

            mv = {}

            def issue_mv(q):
                mv[q] = movp.tile([128, 2048], bf16, tag="mov", name=f"mv_{q}")
                nc.scalar.dma_start(mv[q][:], mov_ap[:, q * 2048:(q + 1) * 2048])

            # upfront: just what the first matmul pair needs — all on the
            # sync ring, since the scalar (ACT) queue leads with a ~1.3us
            # ACT_TABLE_LOAD that would delay the first weight descriptors
            issue_h0(0, nc.sync)
            issue_h0(1, nc.sync)
            issue_mst(0, 0, nc.sync, split=True)

            # paced issue schedule: (r, mp) -> thunks run right after that
            # pair's ACT evacuation is enqueued
            sched = {
                (0, 0): [lambda: issue_mst(0, 1), lambda: issue_h0(2, nc.scalar),
                         lambda: issue_h0(3, nc.scalar)],
                (0, 1): [lambda: issue_h0(4, nc.scalar),
                         lambda: issue_h0(5, nc.scalar),
                         lambda: issue_mst(1, 0)],
                (0, 2): [lambda: issue_h0(6, nc.scalar),
                         lambda: issue_h0(7, nc.scalar),
                         lambda: issue_mst(1, 1)],
                (0, 3): [lambda: issue_mv(0), lambda: issue_mv(4)],
                (1, 1): [lambda: issue_mst(2, 0), lambda: issue_mv(1)],
                (1, 2): [lambda: issue_mst(2, 1), lambda: issue_mv(5)],
                (2, 1): [lambda: issue_mst(3, 0), lambda: issue_mv(2)],
                (2, 2): [lambda: issue_mst(3, 1), lambda: issue_mv(6)],
                (3, 1): [lambda: issue_mv(3), lambda: issue_mv(7)],
            }

            H9 = {}

            def emit_pq(r, q, bb):
                """One phase-2 psum group: L10+L11 + out-transpose for quad q
                (of residue r), batch block bb."""
                pq = psp2.tile([128, 512], f32, tag="ps2", name=f"pq_{q}_{bb}")
                for si in range(4):
                    s = q + 8 * si
                    nc.tensor.matmul(
                        pq[:], H9[s][:, bb * 128:(bb + 1) * 128],
                        mv[q][:, si * 512:(si + 1) * 512],
                        start=(si == 0), stop=(si == 3),
                    )
                piece = piecep.tile([128, 512], f32, tag="piece")
                if (q + bb) % 2 == 0:
                    nc.scalar.copy(piece[:], pq[:])
                else:
                    nc.vector.tensor_copy(piece[:], pq[:])
                dst = out_ap[bb * 128:(bb + 1) * 128, :].rearrange(
                    "p (k t c) -> p k t c", k=4, t=8, c=128
                )[:, :, q, :]
                src = piece[:].rearrange("p (k c) -> p k c", k=4, c=128)
                nc.sync.dma_start(dst, src)

            for r in range(4):
                # phase2(r-1) psum groups, interleaved 2-per-pair below so
                # the PE stream never has a residue-boundary gap
                pq2 = (
                    [(q, bb) for bb in range(4) for q in (r - 1, r + 3)]
                    if r >= 1 else []
                )
                # ---- phase 1 + L9 in partner pairs (m, m^1): evacuate ONE
                # psum of each pair to SBUF (ACT), then both stt ops read
                # one PSUM + one SBUF operand (PSUM has a single DVE port)
                for mp in range(4):
                    p1pair = {}
                    for m in (2 * mp, 2 * mp + 1):
                        t = 4 * m + r
                        p1 = psp1.tile([128, 512], f32, tag="ps1", name=f"p1_{t}")
                        for ji in range(4):
                            nc.tensor.matmul(
                                p1[:],
                                mst_slice(r, m, ji),
                                H0cat[m][:, (r ^ ji) * 512:((r ^ ji) + 1) * 512],
                                start=(ji == 0), stop=(ji == 3),
                            )
                        p1pair[m] = p1
                    m0, m1 = 2 * mp, 2 * mp + 1
                    t0, t1 = 4 * m0 + r, 4 * m1 + r
                    ep = chainp.tile([128, 512], bf16, tag="ch", name=f"Ep_{t1}")
                    nc.scalar.copy(ep[:], p1pair[m1][:])
                    H9[t0] = chainp.tile(
                        [128, 512], bf16, tag="ch", name=f"H9_{t0}"
                    )
                    nc.vector.scalar_tensor_tensor(
                        H9[t0][:], ep[:], rb9col(t0), p1pair[m0][:],
                        op0=mult, op1=add,
                    )
                    H9[t1] = chainp.tile(
                        [128, 512], bf16, tag="ch", name=f"H9_{t1}"
                    )
                    nc.vector.scalar_tensor_tensor(
                        H9[t1][:], p1pair[m0][:], rb9col(t1), ep[:],
                        op0=mult, op1=add,
                    )
                    for thunk in sched.get((r, mp), []):
                        thunk()
                    for q, bb in pq2[2 * mp:2 * mp + 2]:
                        emit_pq(r - 1, q, bb)
            for bb in range(4):
                for q in (3, 7):
                    emit_pq(3, q, bb)

    nc.compile()
    return nc


def kernel(x, values, idx_in, idx_out):
    global LAST_EXEC_NS
    from concourse.bass_utils import run_bass_kernel_spmd

    x = np.asarray(x, np.float32)
    assert x.shape == (BATCH, N), x.shape
    mst, tabs, movd = _host_precompute(values, idx_in, idx_out)
    xT = np.ascontiguousarray(x.T.astype(ml_dtypes.bfloat16))

    if "prog" not in _PROGRAM_CACHE:
        _PROGRAM_CACHE["prog"] = _build_program()
    nc = _PROGRAM_CACHE["prog"]

    in_maps = [
        {
            "xT": np.ascontiguousarray(xT[:, i * BSH:(i + 1) * BSH]),
            "mst": mst,
            "tabs": tabs,
            "movd": movd,
        }
        for i in range(NCORES)
    ]
    res = run_bass_kernel_spmd(nc, in_maps, core_ids=list(range(NCORES)))
    if os.environ.get("BENES_TRACE"):
        tres = run_bass_kernel_spmd(
            nc, in_maps, core_ids=list(range(NCORES)), trace=True
        )
        LAST_EXEC_NS = tres.exec_time_ns
        _PROGRAM_CACHE["profile_json"] = tres.profile_json
    out = np.empty((BATCH, N), np.float32)
    for i in range(NCORES):
        out[i * BSH:(i + 1) * BSH] = res.results[i]["out"]
    return out
